# revision 25
# baseline (speedup 1.0000x reference)
"""Trainium2 Bass kernel for nn_CapsuleLayer_45148696216021.

Mathematical structure (verified against the reference):
  caps = einsum('bi,nio->bno', x, rel_W) + rel_b          [B, N, O]
  caps_t[b] = caps[b].T.reshape(N, O)  (torch view quirk)
  u_hat[b,i,n] = sum_o caps_t[b,n,o] * rw[b,i,o]
  Dynamic routing with b_logits starting at 0: softmax over the capsule
  axis of a tensor whose rows (capsule axis) are identical stays exactly
  uniform (1/N) at EVERY iteration, because the agreement update
  b += einsum('bik,bjk->bji', u_hat, v) is j-independent when v rows are
  identical.  Hence the output v[b,j,:] == squash(sum_i u_hat[b,i,:]/N)
  for all j (bitwise identical rows in the reference too).

  sum_i u_hat[b,i,n] = sum_o caps_t[b,n,o] * rwsum[b,o]
  with rwsum[b,o] = sum_i rw[b,i,o].  Substituting the caps_t view:
  su[b,n] = sum_{r,m} caps[b,r,8n+m] * rwsum[b, m*128+r]

  So the only heavy compute is caps = x @ rel_W (34 GFLOP over the
  weights), followed by a cheap weighted reduction.  rwsum and the rel_b
  bias contribution are tiny and computed on the host.

Sharding: the O axis (1024) is split into 8 slices of 128 columns; core d
computes caps[:, :, 128d:128d+128] for all relations, then reduces with
the rwsum weights to su[:, 16d:16d+16] fully on-chip (capsule n uses
exactly caps columns 8n..8n+7, which lie entirely in one slice).  The
only device output is su (8 KB/core); host applies bias + squash +
row-broadcast to the [128,128,128] output.

Precision: W is stored as float8_e3m4 (4 mantissa bits, ~1.3% RMS
per-element error; rel err ~1.36e-2 against the 2e-2 gate).  x stays
bf16 as the stationary matmul operand.  W uses one exact global scale,
folded into the fp32 rwsv multiplier.

Timeline (per the NTFF trace): ~6.2us fixed engine-start preamble, then
weight DMAs issue immediately on all three queues (sync/scalar HWDGE +
gpsimd SWDGE).  A short filler-matmul burst on a memset tile keeps the
PE busy from its first possible cycle so the HAM clock gate reaches
2.4 GHz right as the first weight chunks land (~10.5us).  The head of
the schedule uses 1- and 2-relation chunks so real matmuls start as
early as possible; steady state streams 4-relation groups whose per-part
weighted reduction (DVE mult from PSUM, Pool accumulate) trails the PE.
The tail pre-reduces the accumulator on Pool during the final group,
fuses the last two 2-relation parts straight into su on the DVE, and
drains su through the scalar queue.
"""

import os
import sys
import tempfile
from concurrent.futures import ThreadPoolExecutor

import numpy as np
import ml_dtypes

if "/opt/trn_rl_repo" not in sys.path:
    sys.path.insert(0, "/opt/trn_rl_repo")

import concourse.bass as bass
import concourse.mybir as mybir
import concourse.tile as tile
from concourse.vector_clock import ScopedClock
from concourse import bass_utils
from concourse.bass_utils import run_bass_kernel_spmd


def _ensure_ntff_hook():
    """This image's antenv lacks axon_hooks, so trace=True dies on import.
    Recreate the module and register the ctypes NTFF hook exactly as
    trn_agent_boot would have (silent no-op when the real module exists)."""
    try:
        import antenv.axon_hooks  # noqa: F401
        return
    except ImportError:
        pass
    try:
        import types

        import antenv
        from trn_agent_boot.trn_boot import _ntff_profile_via_ctypes

        hook = _ntff_profile_via_ctypes("/opt/axon/libaxon_pjrt.so")
        mod = types.ModuleType("antenv.axon_hooks")
        _h = [hook]
        mod.get_axon_ntff_profile_hook = lambda: _h[0]
        mod.set_axon_ntff_profile_hook = lambda h: _h.__setitem__(0, h)
        sys.modules["antenv.axon_hooks"] = mod
        antenv.axon_hooks = mod
    except Exception:
        pass


_ensure_ntff_hook()

_orig_upload = bass_utils.upload_artifacts


def _safe_upload(tmpdir):
    try:
        return _orig_upload(tmpdir)
    except Exception:
        return tmpdir


bass_utils.upload_artifacts = _safe_upload

B, I, O, N = 128, 1024, 1024, 128
NC = 8          # cores
CSL = O // NC   # 128 c-columns per core

LAST_RESULTS = None  # stashed BassKernelResults for test.py introspection


def _cheap_tail(self, tick_clock, wait_clock):
    """Minimal Tile kernel tail: observe the global clock via NOP wait
    chains DISTRIBUTED across all five engines (so the serial chain on any
    one engine is ~5x shorter and the walrus end-barrier fires sooner).
    Semaphore zeroing is left to the walrus codegen epilogue, which
    blanket-clears the whole sem window after its end barrier anyway.
    No drains / all-engine barriers: every proc's final tick is in the
    global clock, so nothing can touch a semaphore afterwards."""
    from concourse.vector_clock import VectorClock

    gc = list(tick_clock.global_clock)
    engines = [
        self.nc.gpsimd,
        self.nc.vector,
        self.nc.scalar,
        self.nc.sync,
        self.nc.tensor,
    ]
    for i, eng in enumerate(engines):
        sub = [t if j % len(engines) == i else 0 for j, t in enumerate(gc)]
        if not any(sub):
            continue
        carrier = eng.nop(nofuse=True)
        wait_clock.add_sem_waits(
            carrier.ins, ScopedClock({None: VectorClock(sub)})
        )
    popped = self.nc._tile_sem_poison_stack.pop()
    assert popped is self._sem_poison
    # mark the sems free in bass state without emitting clear instructions
    sems = list(self.sems.allocated().values())
    sem_nums = [s.num if hasattr(s, "num") else s for s in sems]
    self.nc._state.prepend_free_semaphores(sem_nums)
    for poison_set in self.nc._tile_sem_poison_stack:
        poison_set.update(sem_nums)


tile.TileContext._drain_and_barrier = _cheap_tail


def _strip_framework_overhead(nc):
    """Remove the bass preamble all-engine barrier + per-engine drains (a
    single-shot kernel reading no const-APs doesn't need them).  The
    reset-sema drain / range-clear of the tail is kept for re-execution."""
    n = 0
    for f in nc.m.functions:
        for blk in f.blocks:
            keep = []
            for inst in blk.instructions:
                tn = type(inst).__name__
                drop = False
                if tn == "InstDrain" and inst.reset_range_start is None:
                    drop = True
                elif tn == "InstEventSemaphore" and inst.name.startswith(
                    "barrier_"
                ):
                    drop = True
                if drop:
                    n += 1
                else:
                    keep.append(inst)
            blk.instructions = keep
    return n


def _split_multi_waits(nc):
    """This walrus build only supports one semaphore wait per instruction.
    Tile's wait-assigner can attach several; split the extras onto
    same-engine NOPs inserted immediately before the instruction (same
    semantics: the engine blocks on each wait in turn)."""
    n_split = 0
    for f in nc.m.functions:
        for blk in f.blocks:
            new = []
            dirty = False
            for inst in blk.instructions:
                si = inst.sync_info
                waits = list(si.on_wait) if si is not None else []
                if len(waits) > 1:
                    dirty = True
                    n_split += 1
                    for w in waits[:-1]:
                        nop = mybir.InstNoOp(
                            name=nc.get_next_instruction_name(), ins=[], outs=[]
                        )
                        nop.engine = inst.engine
                        nop.sync_info = mybir.SyncInfo(on_wait=[w], on_update=[])
                        new.append(nop)
                    inst.sync_info = mybir.SyncInfo(
                        on_wait=[waits[-1]], on_update=list(si.on_update)
                    )
                new.append(inst)
            if dirty:
                blk.instructions = new
    return n_split


_NC_CACHE = {}
_F_PRE = int(os.environ.get("BASS_F_PRE", "20"))

# Per-queue DMA issue plans (FIFO order).  Entries: ('xt_a',)/('xt_b',)
# for the two bf16 x halves, ('rw', h) for an rwsv half, ('w', lo, hi)
# for relations [lo, hi).  Queues: scalar + sync are HWDGE, gp is SWDGE.
# Built by a local-search solver against MEASURED competing ring shares
# (scalar ~120 GB/s, gp ~124, sync only ~86 -- the three rings share the
# ~330 GB/s per-core HBM budget unevenly), ring starts ~8.0/9.0/9.7us.
# Small 1- and 2-rel chunks at the head so real matmuls begin ~10.5us;
# the final two 2-rel tail parts ride the fast queues.
_ISSUE = {
    'scalar': [('xt_a',), ('w', 1, 2), ('w', 2, 4), ('w', 4, 6), ('w', 10, 12),
               ('w', 16, 20), ('w', 48, 52), ('w', 56, 60), ('w', 64, 68),
               ('w', 76, 80), ('w', 84, 88), ('w', 88, 92), ('w', 100, 104),
               ('w', 104, 108), ('w', 118, 120), ('w', 120, 122),
               ('w', 126, 128)],
    'sync': [('rw', 0), ('rw', 1), ('w', 0, 1), ('w', 6, 8), ('w', 8, 10),
             ('w', 24, 28), ('w', 36, 40), ('w', 60, 64), ('w', 92, 96),
             ('w', 112, 116), ('w', 124, 126)],
    'gp': [('xt_b',), ('w', 12, 16), ('w', 20, 24), ('w', 28, 32),
           ('w', 32, 36), ('w', 40, 44), ('w', 44, 48), ('w', 52, 56),
           ('w', 68, 72), ('w', 72, 76), ('w', 80, 84), ('w', 96, 100),
           ('w', 108, 112), ('w', 116, 118), ('w', 122, 124)],
}

# chunk-size -> count, for exact-size W tile slots (one slot per chunk)
_N_CHUNKS = {1: 2, 2: 11, 4: 26}

# Part consumption order = modeled arrival order (sync arrivals carry a
# +0.9us pessimism so a lagging sync ring can't head-of-line-block the
# PE).  The last two entries are the tail parts that bypass the
# accumulator; _ORDER[-3] is the final accumulator-writing part.
_ORDER = [(1, 2), (12, 16), (2, 4), (4, 6), (20, 24), (0, 1), (10, 12),
          (28, 32), (6, 8), (16, 20), (32, 36), (8, 10), (40, 44), (48, 52),
          (44, 48), (56, 60), (24, 28), (52, 56), (64, 68), (68, 72),
          (36, 40), (72, 76), (76, 80), (80, 84), (84, 88), (60, 64),
          (96, 100), (88, 92), (108, 112), (116, 118), (92, 96), (122, 124),
          (100, 104), (104, 108), (112, 116), (118, 120), (120, 122),
          (124, 126), (126, 128)]


def _build_bass():
    """Per-core program: caps matmul over this core's c-slice + weighted
    reduction to su[16 local capsules, 128 b] (n-major for a wide DMA)."""
    key = "v4"
    if key in _NC_CACHE:
        return _NC_CACHE[key]

    f32 = mybir.dt.float32
    f8 = mybir.dt.float8e3
    bf16 = mybir.dt.bfloat16
    nc = bass.Bass("TRN2", target_bir_lowering=False)
    xt_d = nc.declare_dram_parameter("xt", [128, 8, 128], bf16, isOutput=False)
    w_d = nc.declare_dram_parameter("w", [128, 128, 8, CSL], f8, isOutput=False)
    rw_d = nc.declare_dram_parameter("rwsv", [128, 2, 8, 64], f32, isOutput=False)
    # two output pieces, summed on the host: sua = all parts but the
    # last (drained early, hidden under the last part's matmuls), sub =
    # the last part's contribution (the only end-of-kernel DMA).
    sua_d = nc.declare_dram_parameter("sua", [128, 16], f32, isOutput=True)
    sub_d = nc.declare_dram_parameter("sub", [128, 16], f32, isOutput=True)

    dma_eng = {}

    with tile.TileContext(nc) as tc:
        with (
            tc.tile_pool(name="const", bufs=1) as cpool,
            tc.tile_pool(name="wts", bufs=1) as wpool,
            tc.tile_pool(name="tmpv", bufs=3) as tvpool,
            tc.tile_pool(name="ps", bufs=7, space="PSUM") as pspool,
            tc.tile_pool(name="warmp", bufs=1, space="PSUM") as warmpool,
        ):
            dma_eng = {'sync': nc.sync, 'scalar': nc.scalar, 'gp': nc.gpsimd}
            # Warmup source is a memset tile: fillers must not depend on
            # any DMA, so the PE p-state ramp overlaps the first transfers.
            wsrc = cpool.tile([128, 512], bf16)
            nc.vector.memset(wsrc[:], 1.0)

            xt = cpool.tile([128, 8, 128], bf16)
            rw = cpool.tile([128, 2, 8, 64], f32)
            acc = cpool.tile([128, 4, 16, 8], f32)
            nc.vector.memset(acc[:], 0.0)
            su_acc = cpool.tile([128, 16], f32)

            # Emit the dma_starts.  Per-queue FIFO order follows _ISSUE;
            # the global emission interleaves queues by modeled start so
            # Tile's priority order matches the intended timeline.
            chunk_tiles = {}
            merged = []
            rates = {'scalar': 105.0, 'sync': 73.0, 'gp': 140.0}
            t_q = {'scalar': 0.0, 'sync': 0.0, 'gp': 0.0}
            for q, plan in _ISSUE.items():
                for e in plan:
                    nbytes = {'xt_a': 131072, 'xt_b': 131072, 'rw': 262144}.get(
                        e[0], (e[2] - e[1]) * 131072 if e[0] == 'w' else 0
                    )
                    merged.append((t_q[q], q, e))
                    t_q[q] += nbytes / rates[q]
            merged.sort(key=lambda x: x[0])
            for _, q, e in merged:
                eng = dma_eng[q]
                if e[0] == 'xt_a':
                    eng.dma_start(xt[:, 0:4], xt_d[:, 0:4])
                elif e[0] == 'xt_b':
                    eng.dma_start(xt[:, 4:8], xt_d[:, 4:8])
                elif e[0] == 'rw':
                    h = e[1]
                    eng.dma_start(rw[:, h], rw_d[:, h])
                else:
                    lo, hi = e[1], e[2]
                    nr = hi - lo
                    wt = wpool.tile(
                        [128, nr, 8, CSL], f8, tag=f"wt{nr}",
                        bufs=_N_CHUNKS[nr], name=f"wt_{lo}_{hi}",
                    )
                    eng.dma_start(wt[:], w_d[:, lo:hi])
                    chunk_tiles[(lo, hi)] = wt

            pr_last = cpool.tile([128, 16], f32)

            # Scratch psum bank for PE-warming filler matmuls (results
            # unused): covers engine-preamble + first-chunk DMA latency
            # and brings the HAM clock gate to 2.4 GHz before real work.
            warm = warmpool.tile([128, 256], f32, tag="warm")
            for _ in range(_F_PRE):
                nc.tensor.matmul(warm[:], wsrc[:, 0:128], wsrc[:, 0:256])

            n_parts = len(_ORDER)
            for pi, (lo, hi) in enumerate(_ORDER):
                nr = hi - lo
                wt = chunk_tiles[(lo, hi)]
                aoff = lo % 4
                ps = pspool.tile([128, 4, 16, 8], f32, tag="ps")
                for k in range(8):
                    nc.tensor.matmul(
                        ps[:, 0:nr],
                        xt[:, k, :],
                        wt[:, :, k, :],
                        start=(k == 0),
                        stop=(k == 7),
                    )
                # tmp = ps * rwsv[b, m, rel] (broadcast over the capsule
                # axis) on DVE (the only engine here that reads PSUM);
                # acc += tmp on Pool (SBUF-only), pipelining across
                # engines.  The final three parts instead reduce on DVE
                # straight to [128,16] so no big reduction survives past
                # the last matmul (the extra DVE reduce work is kept off
                # the steady state to stay clear of the P0 power wall).
                rsl = rw[:, lo // 64, :, lo % 64:lo % 64 + nr].transpose([0, 2, 1])
                in1 = rsl[:, :, None, :].to_broadcast([128, nr, 16, 8])
                tmp = tvpool.tile([128, 4, 16, 8], f32, tag="tmp")
                nc.vector.tensor_tensor(
                    tmp[:, 0:nr], ps[:, 0:nr], in1, mybir.AluOpType.mult
                )
                if pi < n_parts - 6:
                    nc.gpsimd.tensor_tensor(
                        acc[:, aoff:aoff + nr],
                        acc[:, aoff:aoff + nr],
                        tmp[:, 0:nr],
                        mybir.AluOpType.add,
                    )
                    if pi == n_parts - 7:
                        # all acc parts done: one big DVE reduce, absorbed
                        # under the last three parts' matmul stream
                        nc.vector.tensor_reduce(
                            su_acc[:],
                            acc[:].transpose([0, 2, 1, 3]),
                            mybir.AxisListType.XY,
                            mybir.AluOpType.add,
                        )
                else:
                    pr = pr_last if pi == n_parts - 1 else tvpool.tile(
                        [128, 16], f32, tag="pr"
                    )
                    nc.vector.tensor_reduce(
                        pr[:],
                        tmp[:, 0:nr].transpose([0, 2, 1, 3]),
                        mybir.AxisListType.XY,
                        mybir.AluOpType.add,
                    )
                    if pi < n_parts - 1:
                        nc.gpsimd.tensor_tensor(
                            su_acc[:], su_acc[:], pr[:], mybir.AluOpType.add
                        )
                    if pi == n_parts - 2:
                        # all-but-last contribution: drain early on sync so
                        # its DMA latency hides under the last part's MMs
                        nc.sync.dma_start(sua_d[:], su_acc[:])

            nc.scalar.dma_start(sub_d[:], pr_last[:])

    if os.environ.get("BASS_STRIP_FRAMEWORK", "1") == "1":
        _strip_framework_overhead(nc)
    _split_multi_waits(nc)
    _NC_CACHE[key] = nc
    return nc


_LUT_E3M4 = None


def _to_e3m4(a_f32):
    """Fast float32 -> float8_e3m4 via fp16 + 64K LUT (ml_dtypes astype on
    large arrays is slow; the LUT gather is ~10x faster).  Double rounding
    through fp16 is negligible vs the e3m4 quantization itself."""
    global _LUT_E3M4
    if _LUT_E3M4 is None:
        all16 = np.arange(65536, dtype=np.uint16).view(np.float16)
        _LUT_E3M4 = (
            all16.astype(np.float32).astype(ml_dtypes.float8_e3m4).view(np.uint8)
        )
    h = np.ascontiguousarray(a_f32, np.float32).astype(np.float16).view(np.uint16)
    return _LUT_E3M4[h].view(ml_dtypes.float8_e3m4)


def _to_bf16(a):
    """Fast float32 -> bfloat16 with round-to-nearest-even (numpy bit ops;
    ml_dtypes astype is ~50x slower)."""
    u = np.ascontiguousarray(a, np.float32).view(np.uint32)
    r = ((u >> 16) & 1) + np.uint32(0x7FFF)
    return ((u + r) >> 16).astype(np.uint16).view(ml_dtypes.bfloat16)


def _prep_core_w(w8, d):
    # w8: [128, 8, 128, NC, CSL] = (rel, k, i_loc, d, c) uint8 view of
    # quantized rel_W -> per-core [i_loc, rel, k, c]
    return np.ascontiguousarray(
        w8[:, :, :, d, :].transpose(2, 0, 1, 3)
    ).view(ml_dtypes.float8_e3m4)


def kernel(x, edge_index, edge_type, rel_W, rel_b, route_weights):
    global LAST_RESULTS
    x = np.asarray(x, np.float32)
    rel_W = np.asarray(rel_W, np.float32)
    rel_b = np.asarray(rel_b, np.float32)
    rw = np.asarray(route_weights, np.float32).reshape(B, I, O)

    # host-side tiny reductions
    rwsum = rw.sum(axis=1, dtype=np.float32)                # [B, O]
    rwsv = np.ascontiguousarray(rwsum.reshape(B, 8, 128))   # [b, m, r]
    bias2 = np.einsum(
        "rnm,bmr->bn", rel_b.reshape(N, N, 8), rwsv, optimize=True
    )  # [B, N]

    # x stays bf16 (stationary operand; e3m4 for x fails the error gate)
    xt = np.ascontiguousarray(
        _to_bf16(x).view(np.uint16).reshape(B, 8, 128).transpose(2, 1, 0)
    ).view(ml_dtypes.bfloat16)  # [i_loc, k, b]

    # quantize W with one exact global scale placing |W|max near e3m4 top
    wscale = np.float32(15.0 / np.abs(rel_W).max())
    w8 = _to_e3m4(rel_W * wscale).view(np.uint8)
    w8 = w8.reshape(N, 8, 128, NC, CSL)  # (rel, k, i_loc, d, c)
    with ThreadPoolExecutor(NC) as ex:
        w_cores = list(ex.map(lambda d: _prep_core_w(w8, d), range(NC)))

    # fold the W quantization scale into the rwsv multiplier (exact in
    # fp32, then bf16 -- rwsv noise is ~0.3% RMS, far under the W-quant
    # noise), laid out [b, rel_half, m, rel_in_half] so each half is a
    # contiguous per-partition DMA
    rwsv_adj = rwsv / wscale
    rwsv_dev = np.ascontiguousarray(
        rwsv_adj.transpose(0, 2, 1).reshape(B, 2, 64, 8).transpose(0, 1, 3, 2)
    )  # [b, half, m, r%64]

    nc = _build_bass()
    in_maps = [
        {"xt": xt, "w": w_cores[d], "rwsv": rwsv_dev} for d in range(NC)
    ]
    trace = bool(int(os.environ.get("KERNEL_TRACE", "0")))
    kwargs = {}
    if trace:
        kwargs["tmpdir"] = os.environ.get("KERNEL_TRACE_DIR") or tempfile.mkdtemp(
            prefix="capsule_trace_"
        )
    res = run_bass_kernel_spmd(nc, in_maps, list(range(NC)), trace=trace, **kwargs)
    LAST_RESULTS = res

    su = np.concatenate(
        [res.results[d]["sua"] + res.results[d]["sub"] for d in range(NC)],
        axis=1,
    )  # [B, N]
    su += bias2

    s = su * np.float32(1.0 / N)
    sn = np.sum(s * s, axis=-1, keepdims=True)
    vrow = (sn / (1.0 + sn) * s / np.sqrt(sn)).astype(np.float32)  # [B, N]
    out = np.empty((B, N, N), np.float32)
    out[:] = vrow[:, None, :]
    return out
